# revision 1
# baseline (speedup 1.0000x reference)
"""CapsuleLayer dynamic-routing kernel for 8 Trainium2 NeuronCores.

Problem: x[32, 2048, 16], W[1, 2048, 64, 32, 16] -> v[32, 64, 32]
  u_hat = einsum('iodk,bik->biod', W[0], x)
  3 routing iterations (softmax over out_caps, squash over out_dim).

Sharding: in_caps (i) split 8 ways (256/core).  W shard is SBUF-resident in
bf16.  Per routing pass the tensor engine recomputes u_hat tile-by-tile
(16 concurrent small matmuls via 32x32 tile_position packing); the vector
engine applies the V-weighted d-reduction (agreement/logits) and the
exp-weighted moving operand for the selector-matmul that accumulates
s_j = sum_i c_ij * u_hat directly in PSUM.  s_j is AllReduduced across the
8 cores (it is the only cross-core quantity, 256 KB); squash + softmax
bookkeeping is replicated on every core.

Routing state trick: b_ij(t) = sum_d u_hat * (v_0+...+v_{t-1}), so no
b_ij state is carried - only the accumulated V (32x2048 f32).
"""

import os
import numpy as np
import ml_dtypes

B, IC, KD, OC, OD = 32, 2048, 16, 64, 32     # batch, in_caps, in_dim, out_caps, out_dim
NCORES = 8
ICC = IC // NCORES                            # 256 in_caps per core
NJ = ICC // 8                                 # 32 j-blocks (8 i per block)
OD2 = OC * OD                                 # 2048 flattened (o, d)
NUM_ROUTES = 3

_CACHE = {}


def _build_program():
    import concourse.bacc as bacc
    import concourse.tile as tile
    import concourse.mybir as mybir

    f32 = mybir.dt.float32
    bf16 = mybir.dt.bfloat16
    ALU = mybir.AluOpType
    ACTF = mybir.ActivationFunctionType

    nc = bacc.Bacc("TRN2", target_bir_lowering=False, debug=False, num_devices=NCORES)

    WL_d = nc.dram_tensor("WL", [128, NJ * OD2], bf16, kind="ExternalInput").ap()
    xS0_d = nc.dram_tensor("xS0", [128, NJ * B], bf16, kind="ExternalInput").ap()
    xS1_d = nc.dram_tensor("xS1", [128, NJ * B], bf16, kind="ExternalInput").ap()
    SEL1_d = nc.dram_tensor("SEL1", [128, 32], bf16, kind="ExternalInput").ap()
    X2_d = nc.dram_tensor("X2", [128, NJ * B], bf16, kind="ExternalInput").ap()
    vout_d = nc.dram_tensor("v_out", [B, OD2], f32, kind="ExternalOutput").ap()

    with tile.TileContext(nc) as tc:
        with (
            tc.tile_pool(name="const", bufs=1) as cp,
            tc.tile_pool(name="work", bufs=2) as wp,
            tc.tile_pool(name="small", bufs=2) as sp,
            tc.tile_pool(name="psum", bufs=1, space="PSUM") as pp,
            tc.tile_pool(name="dram", bufs=1, space="DRAM") as dp,
        ):
            # ---- resident inputs ----
            wl = cp.tile([128, NJ * OD2], bf16, tag="wl")
            for blk in range(8):
                w = NJ * OD2 // 8
                nc.sync.dma_start(out=wl[:, blk * w:(blk + 1) * w],
                                  in_=WL_d[:, blk * w:(blk + 1) * w])
            xs = [cp.tile([128, NJ * B], bf16, tag=f"xs{s}", name=f"xs{s}") for s in range(2)]
            nc.sync.dma_start(out=xs[0][:, :], in_=xS0_d[:, :])
            nc.sync.dma_start(out=xs[1][:, :], in_=xS1_d[:, :])
            sel1 = cp.tile([128, 32], bf16, tag="sel1")
            nc.sync.dma_start(out=sel1[:, :], in_=SEL1_d[:, :])
            x2t = cp.tile([128, NJ * B], bf16, tag="x2t")
            nc.sync.dma_start(out=x2t[:, :], in_=X2_d[:, :])

            # ---- persistent state ----
            V4 = cp.tile([128, OD2], f32, tag="V4")    # V replicated x4 part-groups
            Vacc = cp.tile([B, OD2], f32, tag="Vacc")  # running sum of v_t

            ar_in = [dp.tile([B, OD2], f32, tag=f"ari{t}", name=f"ari{t}") for t in range(NUM_ROUTES)]
            ar_out = [dp.tile([B, OD2], f32, tag=f"aro{t}", name=f"aro{t}") for t in range(NUM_ROUTES)]

            def uhat_mms(dst_tiles, jj, s_, start, stop):
                """16 matmuls producing u_hat for i_local = jj*8 + 4*s_ + {0..3}.
                dst_tiles[ch][32r:32r+32, :512] <- u_hat[i(r), b, od-chunk ch]."""
                for ch in range(4):
                    for r in range(4):
                        nc.tensor.matmul(
                            dst_tiles[ch][32 * r:32 * r + 32, :],
                            lhsT=xs[s_][32 * r:32 * r + 32, jj * B:(jj + 1) * B],
                            rhs=wl[32 * r:32 * r + 32,
                                   jj * OD2 + ch * 512: jj * OD2 + (ch + 1) * 512],
                            start=start, stop=stop,
                            tile_position=(32 * r, 32 * r),
                        )

            def allreduce_s(t, src_psum):
                """Evacuate s (psum [32, 2048]) -> allreduce -> s_sb."""
                s_sb = cp.tile([B, OD2], f32, tag="ssb", name=f"s_sb{t}")
                nc.scalar.copy(s_sb[:, :], src_psum[0:B, :])
                nc.sync.dma_start(out=ar_in[t][:, :], in_=s_sb[:, :])
                nc.gpsimd.collective_compute(
                    "AllReduce", ALU.add,
                    replica_groups=[list(range(NCORES))],
                    ins=[ar_in[t].opt()],
                    outs=[ar_out[t].opt()],
                )
                nc.sync.dma_start(out=s_sb[:, :], in_=ar_out[t][:, :])
                return s_sb

            def squash(t, s_sb):
                """v_t = squash(s_sb).  t<2: Vacc += v_t, V4 <- replicate(Vacc).
                t==2: DMA v_t to output."""
                sq = wp.tile([B, OD2], f32, tag="sqv", name=f"sq{t}", bufs=1)
                nc.scalar.activation(sq[:, :], s_sb[:, :], ACTF.Square)
                n2 = sp.tile([B, OC], f32, tag="n2")
                nc.vector.tensor_reduce(
                    n2[:, :], sq[:, :].rearrange("p (o d) -> p o d", d=OD),
                    axis=mybir.AxisListType.X, op=ALU.add)
                r0 = sp.tile([B, OC], f32, tag="r0")
                nc.scalar.activation(r0[:, :], n2[:, :], ACTF.Sqrt)
                # Newton polish: n = 0.5 * (r0 + n2 / r0)
                t1 = sp.tile([B, OC], f32, tag="t1")
                nc.vector.reciprocal(t1[:, :], r0[:, :])
                nc.vector.tensor_mul(t1[:, :], t1[:, :], n2[:, :])
                t2 = sp.tile([B, OC], f32, tag="t2")
                nc.vector.tensor_add(t2[:, :], t1[:, :], r0[:, :])
                nn = sp.tile([B, OC], f32, tag="nn")
                nc.vector.tensor_scalar_mul(nn[:, :], t2[:, :], 0.5)   # |s|
                den = sp.tile([B, OC], f32, tag="den")
                nc.vector.tensor_scalar_add(den[:, :], n2[:, :], 1.0)
                rec = sp.tile([B, OC], f32, tag="rec")
                nc.vector.reciprocal(rec[:, :], den[:, :])
                qq = sp.tile([B, OC], f32, tag="qq")
                nc.vector.tensor_mul(qq[:, :], nn[:, :], rec[:, :])  # |s|/(1+|s|^2)
                vt = wp.tile([B, OD2], f32, tag="sqv", name=f"vt{t}", bufs=1)
                nc.vector.tensor_tensor(
                    out=vt[:, :].rearrange("p (o d) -> p o d", d=OD),
                    in0=s_sb[:, :].rearrange("p (o d) -> p o d", d=OD),
                    in1=qq[:, :].unsqueeze(2).broadcast_to([B, OC, OD]),
                    op=ALU.mult)
                if t == NUM_ROUTES - 1:
                    nc.sync.dma_start(out=vout_d[:, :], in_=vt[:, :])
                else:
                    if t == 0:
                        nc.vector.tensor_copy(Vacc[:, :], vt[:, :])
                    else:
                        nc.vector.tensor_add(Vacc[:, :], Vacc[:, :], vt[:, :])
                    for g in range(4):
                        nc.sync.dma_start(out=V4[32 * g:32 * g + 32, :], in_=Vacc[:, :])

            # ======== pass 1: s0 = sum_i u_hat / 64 ========
            # dense contraction over (i, k): lhsT = x/64 in [(i8,k), b] layout,
            # rhs = W2 [(i8,k), od] streamed from DRAM; 1/8th the matmuls of
            # the per-i form and s0 lands in PSUM directly.
            sacc = pp.tile([B, OD2], f32, tag="sacc")
            for tau in range(NJ):
                for ch in range(4):
                    nc.tensor.matmul(
                        sacc[0:B, ch * 512:(ch + 1) * 512],
                        lhsT=x2t[:, tau * B:(tau + 1) * B],
                        rhs=wl[:, tau * OD2 + ch * 512: tau * OD2 + (ch + 1) * 512],
                        start=(tau == 0), stop=(tau == NJ - 1),
                        tile_position=(0, 0))
            s_sb = allreduce_s(0, sacc)
            squash(0, s_sb)

            # ======== passes 2..3: fused agreement/softmax/s ========
            for t in range(1, NUM_ROUTES):
                sacc = pp.tile([B, OD2], f32, tag="sacc")
                for q in range(2 * NJ):
                    jj, s_ = divmod(q, 2)
                    uh = [pp.tile([128, 512], f32, tag=f"acc{ch}", name=f"uh{t}_{q}_{ch}") for ch in range(4)]
                    uhat_mms(uh, jj, s_, start=True, stop=True)
                    # scalar engine evacuates u_hat to SBUF: frees the PSUM
                    # banks after ~2us so the PE starts the next quad (stays
                    # HAM-warm) while the DVE consumes this quad from SBUF.
                    uhsb = wp.tile([128, OD2], bf16, tag="uhb", name=f"uhsb{t}_{q}")
                    for ch in range(4):
                        nc.scalar.copy(uhsb[:, ch * 512:(ch + 1) * 512], uh[ch][:, :])
                    tmp = wp.tile([128, OD2], bf16, tag="tmp")
                    H = OD2 // 2
                    nc.vector.tensor_mul(tmp[:, :H], uhsb[:, :H], V4[:, :H])
                    nc.gpsimd.tensor_mul(tmp[:, H:], uhsb[:, H:], V4[:, H:])
                    agr = sp.tile([128, OC], f32, tag="agr")
                    nc.vector.tensor_reduce(
                        agr[:, :], tmp[:, :].rearrange("p (o d) -> p o d", d=OD),
                        axis=mybir.AxisListType.X, op=ALU.add)
                    eB = sp.tile([128, OC], bf16, tag="eB")
                    Zs = sp.tile([128, 1], f32, tag="Zs")
                    # ACT's accum_out yields Z = sum_o exp(agr) for free
                    nc.scalar.activation(eB[:, :], agr[:, :], ACTF.Exp,
                                         accum_out=Zs[:, :])
                    rZ = sp.tile([128, 1], f32, tag="rZ")
                    nc.vector.reciprocal(rZ[:, :], Zs[:, :])
                    tmp2 = wp.tile([128, OD2], bf16, tag="tmp2b", name=f"tmp2b{t}_{q}")
                    nc.vector.scalar_tensor_tensor(
                        out=tmp2[:, :].rearrange("p (o d) -> p o d", d=OD),
                        in0=uhsb[:, :].rearrange("p (o d) -> p o d", d=OD),
                        scalar=rZ[:, :],
                        in1=eB[:, :].unsqueeze(2).broadcast_to([128, OC, OD]),
                        op0=ALU.mult, op1=ALU.mult)
                    for ch in range(4):
                        nc.tensor.matmul(
                            sacc[0:B, ch * 512:(ch + 1) * 512], lhsT=sel1[:, :],
                            rhs=tmp2[:, ch * 512:(ch + 1) * 512],
                            start=(q == 0), stop=(q == 2 * NJ - 1),
                            tile_position=(0, 0))
                s_sb = allreduce_s(t, sacc)
                squash(t, s_sb)

    nc.compile()
    return nc


def _host_inputs(x, W):
    """Build per-core input maps (host-side relayout, not device time)."""
    W0 = np.asarray(W)[0]                       # [IC, OC, OD, KD]
    x = np.asarray(x)                           # [B, IC, KD]
    in_maps = []
    sel1 = np.zeros((128, 32), np.float32)
    for p in range(128):
        sel1[p, p % 32] = 1.0
    for c in range(NCORES):
        # single W layout serving both passes: partition 16*i8 + k, col tau*2048+od,
        # with i_local = 8*tau + i8.  Per-i windows use i8 = 2r + s (window r,
        # sub-slot s) so every K=32 window is 32-aligned.
        Wc = W0[c * ICC:(c + 1) * ICC].reshape(NJ, 8, OD2, KD)      # [tau, i8, od, k]
        WL = np.ascontiguousarray(Wc.transpose(1, 3, 0, 2)          # [i8, k, tau, od]
                                  ).reshape(128, NJ * OD2)
        xc = x[:, c * ICC:(c + 1) * ICC, :].reshape(B, NJ, 8, KD)   # [b, tau, i8, k]
        xss = []
        for s in range(2):
            Xs = np.zeros((4, 2, KD, NJ, B), np.float32)            # [r, s', k, tau, b]
            Xs[:, s] = xc[:, :, s::2].transpose(2, 3, 1, 0)         # [r, k, tau, b]
            xss.append(Xs.reshape(128, NJ * B))
        X2 = (np.ascontiguousarray(xc.transpose(2, 3, 1, 0))        # [i8, k, tau, b]
              .reshape(128, NJ * B) / float(OC))
        in_maps.append({
            "WL": WL.astype(ml_dtypes.bfloat16),
            "xS0": xss[0].astype(ml_dtypes.bfloat16),
            "xS1": xss[1].astype(ml_dtypes.bfloat16),
            "SEL1": sel1.astype(ml_dtypes.bfloat16),
            "X2": X2.astype(ml_dtypes.bfloat16),
        })
    return in_maps


def kernel(x, W, _want_trace=False):
    from concourse.bass_utils import run_bass_kernel_spmd

    if "nc" not in _CACHE:
        _CACHE["nc"] = _build_program()
    nc = _CACHE["nc"]
    in_maps = _host_inputs(x, W)
    res = run_bass_kernel_spmd(nc, in_maps, core_ids=list(range(NCORES)),
                               trace=_want_trace)
    _CACHE["last_result"] = res
    out = np.asarray(res.results[0]["v_out"], np.float32)
    return out.reshape(B, OC, OD)



# revision 9
# speedup vs baseline: 1.0925x; 1.0925x over previous
"""CapsuleLayer dynamic-routing kernel for 8 Trainium2 NeuronCores. v2

Problem: x[32, 2048, 16], W[1, 2048, 64, 32, 16] -> v[32, 64, 32]
  u_hat = einsum('iodk,bik->biod', W[0], x)
  3 routing iterations (softmax over out_caps, squash over out_dim).

Sharding: in_caps split 8 ways (256/core); W resident in SBUF bf16; s_j
AllReduced per routing pass (only cross-core quantity).

v2 design (vs v1): all wide DVE ops run in 2x mode:
 - columns laid out (d, o) with o innermost so the softmax scale e''[p,o]
   broadcasts over the OUTER dim (stride-0 middle) keeping step-1 inner:
   measured 1135ns per [128,2048] bf16 TT (2x).
 - agreement d-reduction = 5-stage pairwise TT-add tree (all step-1 inner).
 - V/Vacc in bf16 so every TT is pure-bf16 (f32 operand forces 1x).
 - no GpSimd elementwise (shares SBUF port with DVE; concurrency stretches
   both ~3-6x - measured).
 - PSUM: 2 x [128,1024] rotating u_hat tiles + [32,2048] s-accumulator = 8
   banks, so the PE never waits on evacuation (v1 serialized here).
 - squash uses sqrt(n2) = exp(0.5*ln(n2)): keeps ACT on one table set
   (natural_log_exp) instead of thrashing exp<->sqrt loads.
"""

import numpy as np
import ml_dtypes

B, IC, KD, OC, OD = 32, 2048, 16, 64, 32
NCORES = 8
ICC = IC // NCORES                            # 256 in_caps per core
NJ = ICC // 8                                 # 32 tau blocks (8 i per block)
OD2 = OC * OD                                 # 2048 flattened cols, (d, o) order
NUM_ROUTES = 3
EPS2 = 1e-12

_CACHE = {}


def _build_program():
    import concourse.bacc as bacc
    import concourse.tile as tile
    import concourse.mybir as mybir

    f32 = mybir.dt.float32
    bf16 = mybir.dt.bfloat16
    ALU = mybir.AluOpType
    ACTF = mybir.ActivationFunctionType

    nc = bacc.Bacc("TRN2", target_bir_lowering=False, debug=False, num_devices=NCORES)

    WL_d = nc.dram_tensor("WL", [128, NJ * OD2], bf16, kind="ExternalInput").ap()
    xS0_d = nc.dram_tensor("xS0", [128, NJ * B], bf16, kind="ExternalInput").ap()
    xS1_d = nc.dram_tensor("xS1", [128, NJ * B], bf16, kind="ExternalInput").ap()
    SEL1_d = nc.dram_tensor("SEL1", [128, 32], bf16, kind="ExternalInput").ap()
    X2_d = nc.dram_tensor("X2", [128, NJ * B], bf16, kind="ExternalInput").ap()
    vout_d = nc.dram_tensor("v_out", [B, OD2], f32, kind="ExternalOutput").ap()

    with tile.TileContext(nc) as tc:
        with (
            tc.tile_pool(name="const", bufs=1) as cp,
            tc.tile_pool(name="work", bufs=2) as wp,
            tc.tile_pool(name="small", bufs=2) as sp,
            tc.tile_pool(name="bound", bufs=1) as bp,
            tc.tile_pool(name="psum", bufs=2, space="PSUM") as pp,
            tc.tile_pool(name="psacc", bufs=1, space="PSUM") as pa,
            tc.tile_pool(name="dram", bufs=1, space="DRAM") as dp,
        ):
            # ---- resident inputs ----
            wl = cp.tile([128, NJ * OD2], bf16, tag="wl")
            for blk in range(8):
                w = NJ * OD2 // 8
                nc.sync.dma_start(out=wl[:, blk * w:(blk + 1) * w],
                                  in_=WL_d[:, blk * w:(blk + 1) * w])
            xs = [cp.tile([128, NJ * B], bf16, tag=f"xs{s}", name=f"xs{s}") for s in range(2)]
            nc.sync.dma_start(out=xs[0][:, :], in_=xS0_d[:, :])
            nc.sync.dma_start(out=xs[1][:, :], in_=xS1_d[:, :])
            sel1 = cp.tile([128, 32], bf16, tag="sel1")
            nc.sync.dma_start(out=sel1[:, :], in_=SEL1_d[:, :])
            x2t = cp.tile([128, NJ * B], bf16, tag="x2t")
            nc.sync.dma_start(out=x2t[:, :], in_=X2_d[:, :])

            # ---- persistent state ----
            V4 = cp.tile([128, OD2], bf16, tag="V4")    # Vacc replicated x4 part-groups
            Vacc = cp.tile([B, OD2], bf16, tag="Vacc")  # running sum of v_t, (d,o) cols

            ar_in = [dp.tile([B, OD2], f32, tag=f"ari{t}", name=f"ari{t}") for t in range(NUM_ROUTES)]
            ar_out = [dp.tile([B, OD2], f32, tag=f"aro{t}", name=f"aro{t}") for t in range(NUM_ROUTES)]

            def allreduce_s(t, src_psum):
                """Evacuate s (psum [32, 2048] f32) -> allreduce -> s_sb f32."""
                s_sb = cp.tile([B, OD2], f32, tag="ssb", name=f"s_sb{t}")
                nc.scalar.copy(s_sb[:, :], src_psum[0:B, :])
                nc.sync.dma_start(out=ar_in[t][:, :], in_=s_sb[:, :])
                nc.gpsimd.collective_compute(
                    "AllReduce", ALU.add,
                    replica_groups=[list(range(NCORES))],
                    ins=[ar_in[t].opt()],
                    outs=[ar_out[t].opt()],
                )
                nc.sync.dma_start(out=s_sb[:, :], in_=ar_out[t][:, :])
                return s_sb

            def squash(t, s_sb):
                """v_t = squash(s_sb) with s in (d,o) cols.
                t<2: Vacc (+)= v_t (bf16), V4 <- replicate(Vacc).
                t==2: DMA v_t (f32) to output with (d,o)->(o,d) reorder."""
                # squared norms per (b, o): n2 = sum_d s^2
                sq = bp.tile([B, OD2], bf16, tag="sqv", name=f"sq{t}")
                nc.scalar.activation(sq[:, :], s_sb[:, :], ACTF.Square)
                sqv = sq[:, :].rearrange("p (d o) -> p d o", o=OC)
                q1 = bp.tile([B, 16 * OC], bf16, tag="q1", name=f"q1_{t}")
                nc.vector.tensor_tensor(out=q1[:, :].rearrange("p (d o) -> p d o", o=OC),
                                        in0=sqv[:, 0:16, :], in1=sqv[:, 16:32, :], op=ALU.add)
                q1v = q1[:, :].rearrange("p (d o) -> p d o", o=OC)
                q2 = bp.tile([B, 8 * OC], bf16, tag="q2", name=f"q2_{t}")
                nc.vector.tensor_tensor(out=q2[:, :].rearrange("p (d o) -> p d o", o=OC),
                                        in0=q1v[:, 0:8, :], in1=q1v[:, 8:16, :], op=ALU.add)
                q2v = q2[:, :].rearrange("p (d o) -> p d o", o=OC)
                q3 = bp.tile([B, 4 * OC], bf16, tag="q3", name=f"q3_{t}")
                nc.vector.tensor_tensor(out=q3[:, :].rearrange("p (d o) -> p d o", o=OC),
                                        in0=q2v[:, 0:4, :], in1=q2v[:, 4:8, :], op=ALU.add)
                q3v = q3[:, :].rearrange("p (d o) -> p d o", o=OC)
                q4 = bp.tile([B, 2 * OC], f32, tag="q4", name=f"q4_{t}")
                nc.vector.tensor_tensor(out=q4[:, :].rearrange("p (d o) -> p d o", o=OC),
                                        in0=q3v[:, 0:2, :], in1=q3v[:, 2:4, :], op=ALU.add)
                q4v = q4[:, :].rearrange("p (d o) -> p d o", o=OC)
                n2 = bp.tile([B, OC], f32, tag="n2", name=f"n2_{t}")
                nc.vector.tensor_tensor(out=n2[:, :], in0=q4v[:, 0:1, :].squeeze(1),
                                        in1=q4v[:, 1:2, :].squeeze(1), op=ALU.add)
                # |s| = sqrt(n2) = exp(0.5 * ln(n2 + eps))  (stays on exp/ln table set)
                lnn = bp.tile([B, OC], f32, tag="lnn", name=f"ln_{t}")
                nc.scalar.activation(lnn[:, :], n2[:, :], ACTF.Ln)
                rt = bp.tile([B, OC], f32, tag="rt", name=f"rt_{t}")
                nc.scalar.activation(rt[:, :], lnn[:, :], ACTF.Exp, scale=0.5)
                den = bp.tile([B, OC], f32, tag="den", name=f"den_{t}")
                nc.vector.tensor_scalar_add(den[:, :], n2[:, :], 1.0)
                rec = bp.tile([B, OC], f32, tag="rec", name=f"rec_{t}")
                nc.vector.reciprocal(rec[:, :], den[:, :])
                qq = bp.tile([B, OC], bf16, tag="qq", name=f"qq_{t}")
                nc.vector.tensor_tensor(out=qq[:, :], in0=rt[:, :], in1=rec[:, :],
                                        op=ALU.mult)  # |s|/(1+n2)
                qbc = qq[:, :].unsqueeze(1).broadcast_to([B, OD, OC])
                sv = s_sb[:, :].rearrange("p (d o) -> p d o", o=OC)
                if t == NUM_ROUTES - 1:
                    # write v in (o,d) order (strided out, 1x op) so the
                    # output DMA is a plain contiguous copy
                    vt = bp.tile([B, OD2], f32, tag="vtf", name="vt_f")
                    nc.vector.tensor_tensor(
                        out=vt[:, :].rearrange("p (o d) -> p d o", d=OD),
                        in0=sv, in1=qbc, op=ALU.mult)
                    nc.sync.dma_start(out=vout_d[:, :], in_=vt[:, :])
                else:
                    vt = bp.tile([B, OD2], bf16, tag="vtb", name=f"vt{t}")
                    nc.vector.tensor_tensor(
                        out=vt[:, :].rearrange("p (d o) -> p d o", o=OC),
                        in0=sv, in1=qbc, op=ALU.mult)
                    if t == 0:
                        nc.vector.tensor_copy(Vacc[:, :], vt[:, :])
                    else:
                        nc.vector.tensor_add(Vacc[:, :], Vacc[:, :], vt[:, :])
                    for g in range(4):
                        nc.sync.dma_start(out=V4[32 * g:32 * g + 32, :], in_=Vacc[:, :])

            # ======== pass 1: s0 = sum_i u_hat / 64 (dense over (i8,k)) ========
            sacc = pa.tile([B, OD2], f32, tag="sacc", name="sacc0")
            for tau in range(NJ):
                for ch in range(4):
                    nc.tensor.matmul(
                        sacc[0:B, ch * 512:(ch + 1) * 512],
                        lhsT=x2t[:, tau * B:(tau + 1) * B],
                        rhs=wl[:, tau * OD2 + ch * 512: tau * OD2 + (ch + 1) * 512],
                        start=(tau == 0), stop=(tau == NJ - 1),
                        tile_position=(0, 0))
            s_sb = allreduce_s(0, sacc)
            squash(0, s_sb)

            # ======== passes 2..3: fused agreement/softmax/s ========
            for t in range(1, NUM_ROUTES):
                sacc = pa.tile([B, OD2], f32, tag="sacc", name=f"sacc{t}")
                for q in range(2 * NJ):
                    jj, s_ = divmod(q, 2)
                    # --- PE: u_hat quad into two 2-bank psum tiles ---
                    uhp = [pp.tile([128, 1024], f32, tag="uhp", name=f"uhp{t}_{q}_{h}")
                           for h in range(2)]
                    for h in range(2):
                        for ch in range(2):
                            col = jj * OD2 + (2 * h + ch) * 512
                            for r in range(4):
                                nc.tensor.matmul(
                                    uhp[h][32 * r:32 * r + 32, ch * 512:(ch + 1) * 512],
                                    lhsT=xs[s_][32 * r:32 * r + 32, jj * B:(jj + 1) * B],
                                    rhs=wl[32 * r:32 * r + 32, col: col + 512],
                                    start=True, stop=True,
                                    tile_position=(32 * r, 32 * r),
                                )
                    # --- ACT: evacuate to bf16 SBUF ---
                    uhsb = wp.tile([128, OD2], bf16, tag="uhb", name=f"uhsb{t}_{q}")
                    for h in range(2):
                        nc.scalar.copy(uhsb[:, h * 1024:(h + 1) * 1024], uhp[h][:, :])
                    # --- DVE: agreement tmp = uhsb * V4 (bf16 TT, 2x) ---
                    tmp = wp.tile([128, OD2], bf16, tag="tmp", name=f"tmp{t}_{q}")
                    nc.vector.tensor_tensor(out=tmp[:, :], in0=uhsb[:, :], in1=V4[:, :],
                                            op=ALU.mult)
                    # --- DVE: 5-stage pairwise tree over d -> agr [128, 64] f32 ---
                    tv = tmp[:, :].rearrange("p (d o) -> p d o", o=OC)
                    t1 = sp.tile([128, 16 * OC], bf16, tag="t1", name=f"t1_{t}_{q}")
                    nc.vector.tensor_tensor(out=t1[:, :].rearrange("p (d o) -> p d o", o=OC),
                                            in0=tv[:, 0:16, :], in1=tv[:, 16:32, :], op=ALU.add)
                    t1v = t1[:, :].rearrange("p (d o) -> p d o", o=OC)
                    t2 = sp.tile([128, 8 * OC], bf16, tag="t2", name=f"t2_{t}_{q}")
                    nc.vector.tensor_tensor(out=t2[:, :].rearrange("p (d o) -> p d o", o=OC),
                                            in0=t1v[:, 0:8, :], in1=t1v[:, 8:16, :], op=ALU.add)
                    t2v = t2[:, :].rearrange("p (d o) -> p d o", o=OC)
                    t3 = sp.tile([128, 4 * OC], bf16, tag="t3", name=f"t3_{t}_{q}")
                    nc.vector.tensor_tensor(out=t3[:, :].rearrange("p (d o) -> p d o", o=OC),
                                            in0=t2v[:, 0:4, :], in1=t2v[:, 4:8, :], op=ALU.add)
                    t3v = t3[:, :].rearrange("p (d o) -> p d o", o=OC)
                    t4 = sp.tile([128, 2 * OC], f32, tag="t4", name=f"t4_{t}_{q}")
                    nc.vector.tensor_tensor(out=t4[:, :].rearrange("p (d o) -> p d o", o=OC),
                                            in0=t3v[:, 0:2, :], in1=t3v[:, 2:4, :], op=ALU.add)
                    t4v = t4[:, :].rearrange("p (d o) -> p d o", o=OC)
                    agr = sp.tile([128, OC], f32, tag="agr", name=f"agr{t}_{q}")
                    nc.vector.tensor_tensor(out=agr[:, :], in0=t4v[:, 0:1, :].squeeze(1),
                                            in1=t4v[:, 1:2, :].squeeze(1), op=ALU.add)
                    # --- ACT: softmax numerator + Z ---
                    eB = sp.tile([128, OC], bf16, tag="eB", name=f"eB{t}_{q}")
                    Zs = sp.tile([128, 1], f32, tag="Zs", name=f"Zs{t}_{q}")
                    nc.scalar.activation(eB[:, :], agr[:, :], ACTF.Exp, accum_out=Zs[:, :])
                    rZ = sp.tile([128, 1], f32, tag="rZ", name=f"rZ{t}_{q}")
                    nc.vector.reciprocal(rZ[:, :], Zs[:, :])
                    epp = sp.tile([128, OC], bf16, tag="epp", name=f"epp{t}_{q}")
                    nc.vector.tensor_scalar_mul(epp[:, :], eB[:, :], rZ[:, :])
                    # --- DVE: tmp2 = uhsb * c (broadcast over outer d: 2x) ---
                    tmp2 = wp.tile([128, OD2], bf16, tag="tmp2", name=f"tmp2_{t}_{q}")
                    nc.vector.tensor_tensor(
                        out=tmp2[:, :].rearrange("p (d o) -> p d o", o=OC),
                        in0=uhsb[:, :].rearrange("p (d o) -> p d o", o=OC),
                        in1=epp[:, :].unsqueeze(1).broadcast_to([128, OD, OC]),
                        op=ALU.mult)
                    # --- PE: s += sel1^T @ tmp2 ---
                    for ch in range(4):
                        nc.tensor.matmul(
                            sacc[0:B, ch * 512:(ch + 1) * 512], lhsT=sel1[:, :],
                            rhs=tmp2[:, ch * 512:(ch + 1) * 512],
                            start=(q == 0), stop=(q == 2 * NJ - 1),
                            tile_position=(0, 0))
                s_sb = allreduce_s(t, sacc)
                squash(t, s_sb)

    nc.compile()
    return nc


def _host_inputs(x, W):
    """Per-core input maps (host-side relayout, not device time)."""
    W0 = np.asarray(W)[0]                       # [IC, OC, OD, KD]
    x = np.asarray(x)                           # [B, IC, KD]
    in_maps = []
    sel1 = np.zeros((128, 32), np.float32)
    for p in range(128):
        sel1[p, p % 32] = 1.0
    for c in range(NCORES):
        # W layout: partition 16*i8 + k, col tau*2048 + d*64 + o  ((d,o) order)
        Wc = (W0[c * ICC:(c + 1) * ICC]
              .reshape(NJ, 8, OC, OD, KD))                      # [tau, i8, o, d, k]
        WL = np.ascontiguousarray(Wc.transpose(1, 4, 0, 3, 2)   # [i8, k, tau, d, o]
                                  ).reshape(128, NJ * OD2)
        xc = x[:, c * ICC:(c + 1) * ICC, :].reshape(B, NJ, 8, KD)   # [b, tau, i8, k]
        xss = []
        for s in range(2):
            Xs = np.zeros((4, 2, KD, NJ, B), np.float32)            # [r, s', k, tau, b]
            Xs[:, s] = xc[:, :, s::2].transpose(2, 3, 1, 0)         # [r, k, tau, b]
            xss.append(Xs.reshape(128, NJ * B))
        X2 = (np.ascontiguousarray(xc.transpose(2, 3, 1, 0))        # [i8, k, tau, b]
              .reshape(128, NJ * B) / float(OC))
        in_maps.append({
            "WL": WL.astype(ml_dtypes.bfloat16),
            "xS0": xss[0].astype(ml_dtypes.bfloat16),
            "xS1": xss[1].astype(ml_dtypes.bfloat16),
            "SEL1": sel1.astype(ml_dtypes.bfloat16),
            "X2": X2.astype(ml_dtypes.bfloat16),
        })
    return in_maps


def kernel(x, W, _want_trace=False):
    from concourse.bass_utils import run_bass_kernel_spmd

    if "nc" not in _CACHE:
        _CACHE["nc"] = _build_program()
    nc = _CACHE["nc"]
    in_maps = _host_inputs(x, W)
    res = run_bass_kernel_spmd(nc, in_maps, core_ids=list(range(NCORES)),
                               trace=_want_trace)
    _CACHE["last_result"] = res
    out = np.asarray(res.results[0]["v_out"], np.float32)
    return out.reshape(B, OC, OD)


# revision 10
# speedup vs baseline: 1.3735x; 1.2572x over previous
"""CapsuleLayer dynamic-routing kernel for 8 Trainium2 NeuronCores. v3

Problem: x[32, 2048, 16], W[1, 2048, 64, 32, 16] -> v[32, 64, 32]
  u_hat = einsum('iodk,bik->biod', W[0], x)
  3 routing iterations (softmax over out_caps, squash over out_dim).

Sharding: in_caps split 8 ways (256/core); W resident in SBUF bf16; s_j
AllReduced per routing pass (only cross-core quantity).

v3 design:
 - columns in (d, o) order, o innermost: the softmax scale e''[p,o]
   broadcasts over the outer d dim keeping step-1 inner -> every wide DVE
   op is a 2x-mode bf16 tensor_tensor (measured ~1.14us per [128,2048]).
 - agreement d-reduction runs on the PE: 32 accumulating identity-matmuls
   (rhs = tmp[:, d, :], lhsT = I) sum the d-slices into an f32 PSUM
   [128,64] tile at 29ns/MM (LDWEIGHTS of the repeated identity pipelines
   through the background weight buffer). Replaces a 2.1us DVE tree.
 - s accumulates in ONE psum bank as [(ch,b), 512] via col-offset
   tile_position selector matmuls (4 col-groups run concurrently, ~330ns).
 - software-pipelined emission: evac(q+1) is queued on ACT before exp(q),
   and mul(q+1) is queued on DVE before recip(q), so neither engine idles
   during the cross-engine ping-pong.
 - no GpSimd elementwise (shares SBUF port with DVE).
 - squash uses sqrt(n2) = exp(0.5*ln(n2)): stays on one ACT table set.
"""

import numpy as np
import ml_dtypes

B, IC, KD, OC, OD = 32, 2048, 16, 64, 32
NCORES = 8
ICC = IC // NCORES                            # 256 in_caps per core
NJ = ICC // 8                                 # 32 tau blocks (8 i per block)
OD2 = OC * OD                                 # 2048 flattened cols, (d, o) order
NQ = 2 * NJ                                   # 64 quads (4 i each)
NUM_ROUTES = 3

_CACHE = {}


def _build_program():
    import concourse.bacc as bacc
    import concourse.tile as tile
    import concourse.mybir as mybir

    f32 = mybir.dt.float32
    bf16 = mybir.dt.bfloat16
    ALU = mybir.AluOpType
    ACTF = mybir.ActivationFunctionType

    nc = bacc.Bacc("TRN2", target_bir_lowering=False, debug=False, num_devices=NCORES)

    WL_d = nc.dram_tensor("WL", [128, NJ * OD2], bf16, kind="ExternalInput").ap()
    xS0_d = nc.dram_tensor("xS0", [128, NJ * B], bf16, kind="ExternalInput").ap()
    xS1_d = nc.dram_tensor("xS1", [128, NJ * B], bf16, kind="ExternalInput").ap()
    SEL1_d = nc.dram_tensor("SEL1", [128, 32], bf16, kind="ExternalInput").ap()
    IDN_d = nc.dram_tensor("IDN", [128, 128], bf16, kind="ExternalInput").ap()
    X2_d = nc.dram_tensor("X2", [128, NJ * B], bf16, kind="ExternalInput").ap()
    vout_d = nc.dram_tensor("v_out", [B, OD2], f32, kind="ExternalOutput").ap()

    with tile.TileContext(nc) as tc:
        with (
            tc.tile_pool(name="const", bufs=1) as cp,
            tc.tile_pool(name="work", bufs=2) as wp,
            tc.tile_pool(name="small", bufs=2) as sp,
            tc.tile_pool(name="bound", bufs=1) as bp,
            tc.tile_pool(name="psum", bufs=2, space="PSUM") as pp,
            tc.tile_pool(name="pagr", bufs=2, space="PSUM") as pg,
            tc.tile_pool(name="psacc", bufs=1, space="PSUM") as pa,
            tc.tile_pool(name="dram", bufs=1, space="DRAM") as dp,
        ):
            # ---- resident inputs ----
            wl = cp.tile([128, NJ * OD2], bf16, tag="wl")
            for blk in range(8):
                w = NJ * OD2 // 8
                nc.sync.dma_start(out=wl[:, blk * w:(blk + 1) * w],
                                  in_=WL_d[:, blk * w:(blk + 1) * w])
            xs = [cp.tile([128, NJ * B], bf16, tag=f"xs{s}", name=f"xs{s}") for s in range(2)]
            nc.sync.dma_start(out=xs[0][:, :], in_=xS0_d[:, :])
            nc.sync.dma_start(out=xs[1][:, :], in_=xS1_d[:, :])
            sel1 = cp.tile([128, 32], bf16, tag="sel1")
            nc.sync.dma_start(out=sel1[:, :], in_=SEL1_d[:, :])
            idn = cp.tile([128, 128], bf16, tag="idn")
            nc.sync.dma_start(out=idn[:, :], in_=IDN_d[:, :])
            x2t = cp.tile([128, NJ * B], bf16, tag="x2t")
            nc.sync.dma_start(out=x2t[:, :], in_=X2_d[:, :])

            # ---- persistent state ----
            V4 = cp.tile([128, OD2], bf16, tag="V4")    # Vacc replicated x4 part-groups
            Vacc = cp.tile([B, OD2], bf16, tag="Vacc")  # running sum of v_t, (d,o) cols

            ar_in = [dp.tile([128, 512], f32, tag=f"ari{t}", name=f"ari{t}") for t in range(NUM_ROUTES)]
            ar_out = [dp.tile([128, 512], f32, tag=f"aro{t}", name=f"aro{t}") for t in range(NUM_ROUTES)]

            def emit_quad(t, q):
                """PE u_hat quad q + ACT evacuation -> uhsb (bf16, (d,o))."""
                jj, s_ = divmod(q, 2)
                uhp = [pp.tile([128, 1024], f32, tag="uhp", name=f"uhp{t}_{q}_{h}")
                       for h in range(2)]
                for h in range(2):
                    for ch in range(2):
                        col = jj * OD2 + (2 * h + ch) * 512
                        for r in range(4):
                            nc.tensor.matmul(
                                uhp[h][32 * r:32 * r + 32, ch * 512:(ch + 1) * 512],
                                lhsT=xs[s_][32 * r:32 * r + 32, jj * B:(jj + 1) * B],
                                rhs=wl[32 * r:32 * r + 32, col: col + 512],
                                start=True, stop=True,
                                tile_position=(32 * r, 32 * r),
                            )
                uhsb = wp.tile([128, OD2], bf16, tag="uhb", name=f"uhsb{t}_{q}")
                for h in range(2):
                    nc.scalar.copy(uhsb[:, h * 1024:(h + 1) * 1024], uhp[h][:, :])
                return uhsb

            def emit_mul(t, q, uhsb):
                """DVE tmp = uhsb * V4 (bf16 2x)."""
                tmp = wp.tile([128, OD2], bf16, tag="tmp", name=f"tmp{t}_{q}")
                nc.vector.tensor_tensor(out=tmp[:, :], in0=uhsb[:, :], in1=V4[:, :],
                                        op=ALU.mult)
                return tmp

            def allreduce_s(t, src_psum):
                """Evacuate packed s (psum [128,512] f32) -> allreduce."""
                s_sb = cp.tile([128, 512], f32, tag="ssb", name=f"s_sb{t}")
                nc.scalar.copy(s_sb[:, :], src_psum[:, :])
                nc.sync.dma_start(out=ar_in[t][:, :], in_=s_sb[:, :])
                nc.gpsimd.collective_compute(
                    "AllReduce", ALU.add,
                    replica_groups=[list(range(NCORES))],
                    ins=[ar_in[t].opt()],
                    outs=[ar_out[t].opt()],
                )
                nc.sync.dma_start(out=s_sb[:, :], in_=ar_out[t][:, :])
                # unpack [(ch,b), 512] -> [32, 2048]
                spk = bp.tile([B, OD2], f32, tag="spk", name=f"spk{t}")
                for ch in range(4):
                    nc.sync.dma_start(out=spk[:, ch * 512:(ch + 1) * 512],
                                      in_=s_sb[32 * ch:32 * ch + 32, :])
                return spk

            def squash(t, s_sb):
                """v_t = squash(s_sb [32,2048] f32, (d,o) cols)."""
                sq = bp.tile([B, OD2], bf16, tag="sqv", name=f"sq{t}")
                nc.scalar.activation(sq[:, :], s_sb[:, :], ACTF.Square)
                sqv = sq[:, :].rearrange("p (d o) -> p d o", o=OC)
                q1 = bp.tile([B, 16 * OC], bf16, tag="q1", name=f"q1_{t}")
                nc.vector.tensor_tensor(out=q1[:, :].rearrange("p (d o) -> p d o", o=OC),
                                        in0=sqv[:, 0:16, :], in1=sqv[:, 16:32, :], op=ALU.add)
                q1v = q1[:, :].rearrange("p (d o) -> p d o", o=OC)
                q2 = bp.tile([B, 8 * OC], bf16, tag="q2", name=f"q2_{t}")
                nc.vector.tensor_tensor(out=q2[:, :].rearrange("p (d o) -> p d o", o=OC),
                                        in0=q1v[:, 0:8, :], in1=q1v[:, 8:16, :], op=ALU.add)
                q2v = q2[:, :].rearrange("p (d o) -> p d o", o=OC)
                q3 = bp.tile([B, 4 * OC], bf16, tag="q3", name=f"q3_{t}")
                nc.vector.tensor_tensor(out=q3[:, :].rearrange("p (d o) -> p d o", o=OC),
                                        in0=q2v[:, 0:4, :], in1=q2v[:, 4:8, :], op=ALU.add)
                q3v = q3[:, :].rearrange("p (d o) -> p d o", o=OC)
                q4 = bp.tile([B, 2 * OC], f32, tag="q4", name=f"q4_{t}")
                nc.vector.tensor_tensor(out=q4[:, :].rearrange("p (d o) -> p d o", o=OC),
                                        in0=q3v[:, 0:2, :], in1=q3v[:, 2:4, :], op=ALU.add)
                q4v = q4[:, :].rearrange("p (d o) -> p d o", o=OC)
                n2 = bp.tile([B, OC], f32, tag="n2", name=f"n2_{t}")
                nc.vector.tensor_tensor(out=n2[:, :], in0=q4v[:, 0:1, :].squeeze(1),
                                        in1=q4v[:, 1:2, :].squeeze(1), op=ALU.add)
                # |s| = sqrt(n2) = exp(0.5*ln(n2)); n2=0 -> qq=0 (exp(-inf))
                lnn = bp.tile([B, OC], f32, tag="lnn", name=f"ln_{t}")
                nc.scalar.activation(lnn[:, :], n2[:, :], ACTF.Ln)
                rt = bp.tile([B, OC], f32, tag="rt", name=f"rt_{t}")
                nc.scalar.activation(rt[:, :], lnn[:, :], ACTF.Exp, scale=0.5)
                den = bp.tile([B, OC], f32, tag="den", name=f"den_{t}")
                nc.vector.tensor_scalar_add(den[:, :], n2[:, :], 1.0)
                rec = bp.tile([B, OC], f32, tag="rec", name=f"rec_{t}")
                nc.vector.reciprocal(rec[:, :], den[:, :])
                qq = bp.tile([B, OC], bf16, tag="qq", name=f"qq_{t}")
                nc.vector.tensor_tensor(out=qq[:, :], in0=rt[:, :], in1=rec[:, :],
                                        op=ALU.mult)  # |s|/(1+n2)
                qbc = qq[:, :].unsqueeze(1).broadcast_to([B, OD, OC])
                sv = s_sb[:, :].rearrange("p (d o) -> p d o", o=OC)
                if t == NUM_ROUTES - 1:
                    # write v in (o,d) order so the output DMA is contiguous
                    vt = bp.tile([B, OD2], f32, tag="vtf", name="vt_f")
                    nc.vector.tensor_tensor(
                        out=vt[:, :].rearrange("p (o d) -> p d o", d=OD),
                        in0=sv, in1=qbc, op=ALU.mult)
                    nc.sync.dma_start(out=vout_d[:, :], in_=vt[:, :])
                else:
                    vt = bp.tile([B, OD2], bf16, tag="vtb", name=f"vt{t}")
                    nc.vector.tensor_tensor(
                        out=vt[:, :].rearrange("p (d o) -> p d o", o=OC),
                        in0=sv, in1=qbc, op=ALU.mult)
                    if t == 0:
                        nc.vector.tensor_copy(Vacc[:, :], vt[:, :])
                    else:
                        nc.vector.tensor_add(Vacc[:, :], Vacc[:, :], vt[:, :])
                    for g in range(4):
                        nc.sync.dma_start(out=V4[32 * g:32 * g + 32, :], in_=Vacc[:, :])

            # ======== pass 1: s0 = sum_i u_hat / 64 (dense over (i8,k)) ========
            sacc = pa.tile([128, 512], f32, tag="sacc", name="sacc0")
            for tau in range(NJ):
                for ch in range(4):
                    nc.tensor.matmul(
                        sacc[32 * ch:32 * ch + 32, :],
                        lhsT=x2t[:, tau * B:(tau + 1) * B],
                        rhs=wl[:, tau * OD2 + ch * 512: tau * OD2 + (ch + 1) * 512],
                        start=(tau == 0), stop=(tau == NJ - 1),
                        tile_position=(0, 32 * ch))
            spk = allreduce_s(0, sacc)
            squash(0, spk)

            # ======== passes 2..3: fused agreement/softmax/s, sw-pipelined ====
            for t in range(1, NUM_ROUTES):
                sacc = pa.tile([128, 512], f32, tag="sacc", name=f"sacc{t}")
                uhsb_next = emit_quad(t, 0)          # prologue: quad 0
                tmp_next = emit_mul(t, 0, uhsb_next)
                for q in range(NQ):
                    uhsb, tmp = uhsb_next, tmp_next
                    if q + 1 < NQ:
                        uhsb_next = emit_quad(t, q + 1)
                    # --- PE: fold tmp over d into f32 agr (32 identity MMs) ---
                    agr = pg.tile([128, 64], f32, tag="agr", name=f"agr{t}_{q}")
                    tv = tmp[:, :].rearrange("p (d o) -> p d o", o=OC)
                    for d in range(OD):
                        nc.tensor.matmul(agr[:, 0:64], lhsT=idn[:, :], rhs=tv[:, d, :],
                                         start=(d == 0), stop=(d == OD - 1),
                                         tile_position=(0, 0))
                    # --- ACT: softmax numerator + Z straight off PSUM ---
                    eB = sp.tile([128, OC], bf16, tag="eB", name=f"eB{t}_{q}")
                    Zs = sp.tile([128, 1], f32, tag="Zs", name=f"Zs{t}_{q}")
                    nc.scalar.activation(eB[:, :], agr[:, 0:64], ACTF.Exp,
                                         accum_out=Zs[:, :])
                    # --- DVE: mul(q+1) fills the fold/exp gap ---
                    if q + 1 < NQ:
                        tmp_next = emit_mul(t, q + 1, uhsb_next)
                    rZ = sp.tile([128, 1], f32, tag="rZ", name=f"rZ{t}_{q}")
                    nc.vector.reciprocal(rZ[:, :], Zs[:, :])
                    epp = sp.tile([128, OC], bf16, tag="epp", name=f"epp{t}_{q}")
                    nc.vector.tensor_scalar_mul(epp[:, :], eB[:, :], rZ[:, :])
                    # --- DVE: tmp2 = uhsb * c (broadcast over outer d: 2x) ---
                    tmp2 = wp.tile([128, OD2], bf16, tag="tmp2", name=f"tmp2_{t}_{q}")
                    nc.vector.tensor_tensor(
                        out=tmp2[:, :].rearrange("p (d o) -> p d o", o=OC),
                        in0=uhsb[:, :].rearrange("p (d o) -> p d o", o=OC),
                        in1=epp[:, :].unsqueeze(1).broadcast_to([128, OD, OC]),
                        op=ALU.mult)
                    # --- PE: s += sel1^T @ tmp2 (4 col-groups, concurrent) ---
                    for ch in range(4):
                        nc.tensor.matmul(
                            sacc[32 * ch:32 * ch + 32, :], lhsT=sel1[:, :],
                            rhs=tmp2[:, ch * 512:(ch + 1) * 512],
                            start=(q == 0), stop=(q == NQ - 1),
                            tile_position=(0, 32 * ch))
                spk = allreduce_s(t, sacc)
                squash(t, spk)

    nc.compile()
    return nc


def _host_inputs(x, W):
    """Per-core input maps (host-side relayout, not device time)."""
    W0 = np.asarray(W)[0]                       # [IC, OC, OD, KD]
    x = np.asarray(x)                           # [B, IC, KD]
    in_maps = []
    sel1 = np.zeros((128, 32), np.float32)
    for p in range(128):
        sel1[p, p % 32] = 1.0
    idn = np.eye(128, dtype=np.float32)
    for c in range(NCORES):
        # W layout: partition 16*i8 + k, col tau*2048 + d*64 + o  ((d,o) order)
        Wc = (W0[c * ICC:(c + 1) * ICC]
              .reshape(NJ, 8, OC, OD, KD))                      # [tau, i8, o, d, k]
        WL = np.ascontiguousarray(Wc.transpose(1, 4, 0, 3, 2)   # [i8, k, tau, d, o]
                                  ).reshape(128, NJ * OD2)
        xc = x[:, c * ICC:(c + 1) * ICC, :].reshape(B, NJ, 8, KD)   # [b, tau, i8, k]
        xss = []
        for s in range(2):
            Xs = np.zeros((4, 2, KD, NJ, B), np.float32)            # [r, s', k, tau, b]
            Xs[:, s] = xc[:, :, s::2].transpose(2, 3, 1, 0)         # [r, k, tau, b]
            xss.append(Xs.reshape(128, NJ * B))
        X2 = (np.ascontiguousarray(xc.transpose(2, 3, 1, 0))        # [i8, k, tau, b]
              .reshape(128, NJ * B) / float(OC))
        in_maps.append({
            "WL": WL.astype(ml_dtypes.bfloat16),
            "xS0": xss[0].astype(ml_dtypes.bfloat16),
            "xS1": xss[1].astype(ml_dtypes.bfloat16),
            "SEL1": sel1.astype(ml_dtypes.bfloat16),
            "IDN": idn.astype(ml_dtypes.bfloat16),
            "X2": X2.astype(ml_dtypes.bfloat16),
        })
    return in_maps


def kernel(x, W, _want_trace=False):
    from concourse.bass_utils import run_bass_kernel_spmd

    if "nc" not in _CACHE:
        _CACHE["nc"] = _build_program()
    nc = _CACHE["nc"]
    in_maps = _host_inputs(x, W)
    res = run_bass_kernel_spmd(nc, in_maps, core_ids=list(range(NCORES)),
                               trace=_want_trace)
    _CACHE["last_result"] = res
    out = np.asarray(res.results[0]["v_out"], np.float32)
    return out.reshape(B, OC, OD)
